# revision 8
# baseline (speedup 1.0000x reference)
"""Trainium2 Bass kernel for the DeepFermi deconvolution GD problem (v2).

Reference: 10 fixed-step GD iterations of a per-pixel objective

    F(eta) = ||ctc_dc - conv(aif_os, fermi_ir(eta))[::8]/8||^2 / C_dc
             + softplus(lambda) * ||(eta - eta_nn)||^2_Cnn + ||relu(-eta)||^2

The time-axis convolution with the fixed AIF is a 64x512 matrix M2 (the sharp
C=500 onset step is folded into it).  The per-pixel factor sigmoid(k*(t0-tsh))
is smooth, so we sample it on an S=64 uniform grid tau and fold the 512->S
linear interpolation into the fixed matrices:

    M2L  = M2 @ L            [64, S]
    M2VL = M2L * tau         [64, S]
    s1_s = sigmoid(k*(t0 - tau_s))     sd_s = s1_s*(1-s1_s)
    q    = M2L @ s1;  qd = M2L @ sd;  qdv = M2VL @ sd
    r2   = (2/C_dc)*(A*q - ctc_dc)
    gA   = r2.q;  U = r2.qd;  V = r2.qdv
    gk   = A*(t0*U - V);  gt0 = A*k*U

(numpy-validated: rel err ~2e-5 vs the 512-point reference, tolerance 2e-2).

Layout: H rows sharded over 8 cores (16 rows = 16 tiles of 128 pixels each).
Time-major [S, pixels] for sigmoid/sd (batched over 4-tile groups), pixel-major
[pixels, j] for the conv outputs.  Conv outputs land in 2-bank quad PSUM tiles
(4 tiles, 256-col pitch) so the PSUM->SBUF copy is one Scalar op per quad and
the dot products are one product op + one segmented tensor_reduce per quad.
"""

import numpy as np

OSAMP = 8
MAX_ITER = 10
NEG_SHIFT = 2 * OSAMP
OTP = 5
C_SHARP = 500.0
LR = 0.1
T = 64
TOS = OSAMP * T  # 512
S = 64           # reduced time-sample grid for the smooth sigmoid
H = 128
W = 128
N_CORES = 8
ROWS_PER_CORE = H // N_CORES  # 16
TILES = ROWS_PER_CORE
P = 128
GROUPS = 4
TPG = TILES // GROUPS  # tiles per group (4)
QPITCH = 256           # per-tile column pitch inside a quad PSUM tile


# ---------------------------------------------------------------------------
# host-side math (iteration independent)
# ---------------------------------------------------------------------------

def _resize_mat(in_size, out_size):
    scale = out_size / in_size
    sample_f = (np.arange(out_size) + 0.5) / scale - 0.5
    x = np.abs(sample_f[None, :] - np.arange(in_size)[:, None])
    w = np.maximum(0.0, 1.0 - x)
    tot = w.sum(0, keepdims=True)
    w = np.where(np.abs(tot) > 1e-4, w / tot, 0.0)
    return w  # float64


def _sigmoid(x):
    return 1.0 / (1.0 + np.exp(-np.clip(x, -500, 500)))


def _preprocess(ctc, aif, time, eta_nn, lambda_reg):
    f64 = np.float64
    R = _resize_mat(T, TOS)
    aif0 = (aif.astype(f64) - aif.astype(f64)[..., :OTP].mean(-1, keepdims=True))
    ctc0 = (ctc.astype(f64) - ctc.astype(f64)[..., :OTP].mean(-1, keepdims=True))
    aif_os = (aif0 @ R)[0, 0, 0]                    # [512]
    t_os = time.astype(f64) @ R                     # [512]
    ctc_dc = (ctc0 @ R[:, ::OSAMP])[0]              # [H,W,64]
    C_dc = float((ctc_dc.astype(np.float32) ** 2).sum(dtype=np.float64))
    tsh = t_os - t_os[NEG_SHIFT]
    s2 = _sigmoid((C_SHARP * tsh).astype(np.float32).astype(f64))
    idx = NEG_SHIFT + 8 * np.arange(T)[:, None] - np.arange(TOS)[None, :]
    valid = (idx >= 0) & (idx <= TOS - 1)
    M = np.where(valid, aif_os[np.clip(idx, 0, TOS - 1)], 0.0) / OSAMP  # [64,512]
    M2 = M * s2[None, :]
    # S-point grid in tsh-space + hat-function interpolation matrix L
    tau = np.linspace(tsh.min(), tsh.max(), S)
    dt_ = tau[1] - tau[0]
    pos = (tsh - tau[0]) / dt_
    i0 = np.clip(np.floor(pos).astype(int), 0, S - 2)
    frac = np.clip(pos - i0, 0.0, 1.0)
    L = np.zeros((TOS, S))
    L[np.arange(TOS), i0] = 1 - frac
    L[np.arange(TOS), i0 + 1] = frac
    M2L = M2 @ L                                    # [64, S]
    M2VL = M2L * tau[None, :]
    C_nn = (eta_nn.astype(f64) ** 2).sum(axis=(0, 2, 3))  # [3]
    sp_lam = np.logaddexp(0.0, float(lambda_reg.reshape(-1)[0]))
    creg = 2.0 * sp_lam / C_nn                      # [3]
    return M2L, M2VL, tau, ctc_dc, C_dc, creg


# ---------------------------------------------------------------------------
# bass module (input-value independent; all data arrives via DRAM tensors)
# ---------------------------------------------------------------------------

_NC_CACHE = {}


def _build_nc():
    if "nc" in _NC_CACHE:
        return _NC_CACHE["nc"]

    import concourse.mybir as mybir
    import concourse.tile as tile
    from concourse import bacc

    dt = mybir.dt.float32
    bf = mybir.dt.bfloat16
    Alu = mybir.AluOpType
    Act = mybir.ActivationFunctionType
    Ax = mybir.AxisListType

    nc = bacc.Bacc("TRN2", target_bir_lowering=False, debug=False)

    # shared constants (identical on every core)
    d_argw = nc.declare_dram_parameter("argw", [2 * TILES, TILES * S], bf,
                                       isOutput=False)
    d_ident = nc.declare_dram_parameter("ident", [P, P], bf, isOutput=False)
    d_m2tl = nc.declare_dram_parameter("m2tl", [S, T], bf, isOutput=False)
    d_muvl = nc.declare_dram_parameter("muvl", [S, 2 * T], bf, isOutput=False)
    d_s48 = nc.declare_dram_parameter("s48", [P, 3 * TILES], dt, isOutput=False)
    d_consts = nc.declare_dram_parameter("consts", [P, TILES], dt, isOutput=False)
    # per-core data
    d_nctc = nc.declare_dram_parameter("negctc2", [P, TILES * T], bf, isOutput=False)
    d_eta0 = nc.declare_dram_parameter("eta0", [P, 3 * TILES], dt, isOutput=False)
    d_cpl48 = nc.declare_dram_parameter("cpl48", [P, 3 * TILES], dt, isOutput=False)
    d_out = nc.declare_dram_parameter("out", [P, 3 * TILES], dt, isOutput=True)

    with tile.TileContext(nc) as tc:
        with (
            tc.tile_pool(name="const", bufs=1) as cpool,
            tc.tile_pool(name="state", bufs=2) as spool,
            tc.tile_pool(name="small", bufs=2) as mpool,
            tc.tile_pool(name="ps_arg", bufs=2, space="PSUM") as ps_arg,
            tc.tile_pool(name="ps_qq", bufs=2, space="PSUM") as ps_qq,
            tc.tile_pool(name="ps_k", bufs=1, space="PSUM") as ps_k,
        ):
            # ---- load constants ----
            argw = cpool.tile([2 * TILES, TILES * S], bf, tag="argw")
            nc.gpsimd.dma_start(argw[:], d_argw[:])
            ident = cpool.tile([P, P], bf, tag="ident")
            nc.gpsimd.dma_start(ident[:], d_ident[:])
            m2tl = cpool.tile([S, T], bf, tag="m2tl")
            nc.gpsimd.dma_start(m2tl[:], d_m2tl[:])
            muvl = cpool.tile([S, 2 * T], bf, tag="muvl")
            nc.gpsimd.dma_start(muvl[:], d_muvl[:])
            nctc = cpool.tile([P, TILES * T], bf, tag="nctc")
            nc.gpsimd.dma_start(nctc[:], d_nctc[:])
            cpl48 = cpool.tile([P, 3 * TILES], dt, tag="cpl48")
            nc.gpsimd.dma_start(cpl48[:], d_cpl48[:])
            s48 = cpool.tile([P, 3 * TILES], dt, tag="s48")
            nc.gpsimd.dma_start(s48[:], d_s48[:])
            consts = cpool.tile([P, TILES], dt, tag="consts")
            nc.gpsimd.dma_start(consts[:], d_consts[:])
            eta_in = cpool.tile([P, 3 * TILES], dt, tag="eta_in")
            nc.gpsimd.dma_start(eta_in[:], d_eta0[:])

            # persistent work buffers
            s1T = cpool.tile([S, TILES * P], bf, tag="s1T")
            sdT = cpool.tile([S, TILES * P], bf, tag="sdT")
            qsall = cpool.tile([P, TILES * 3 * T], bf, tag="qsall")
            r2all = cpool.tile([P, TILES * T], bf, tag="r2all")
            r2tmp = cpool.tile([P, TILES * T], bf, tag="r2tmp")
            prodall = cpool.tile([P, TILES * 3 * T], bf, tag="prodall")
            accAll = cpool.tile([P, 3 * TILES], dt, tag="accAll")
            sdacc = cpool.tile([S, 2], dt, tag="sdacc")

            eta48 = spool.tile([P, 3 * TILES], dt, tag="eta48")
            nc.vector.tensor_copy(eta48[:], eta_in[:])

            for it in range(MAX_ITER):
                eA = eta48[:, 0:TILES]
                eK = eta48[:, TILES:2 * TILES]
                eT = eta48[:, 2 * TILES:3 * TILES]

                # ---- derived per-iteration tensors ----
                kn = spool.tile([P, 2 * TILES], bf, tag="kn")
                nc.gpsimd.tensor_tensor(kn[:, 0:2 * TILES:2], eK, eT, Alu.mult)
                nc.gpsimd.tensor_scalar_mul(kn[:, 1:2 * TILES:2], eK, -1.0)
                knt_ps = ps_k.tile([2 * TILES, P], bf, tag="kntp")
                nc.tensor.transpose(knt_ps[:], kn[:], ident[:])
                knT = spool.tile([2 * TILES, P], bf, tag="knT")
                nc.scalar.copy(knT[:], knt_ps[:])
                # a2c = (2/C_dc) * A
                a2c = spool.tile([P, TILES], dt, tag="a2c")
                nc.gpsimd.tensor_tensor(a2c[:], eA, consts[:], Alu.mult)

                # ---- phase 1: argT -> sigmoid -> sd (4-tile groups) ----
                for g in range(GROUPS):
                    argp = ps_arg.tile([S, TPG * P], dt, tag="argp")
                    for tt in range(TPG):
                        t = g * TPG + tt
                        nc.tensor.matmul(
                            argp[:, tt * P:(tt + 1) * P],
                            argw[:, t * S:(t + 1) * S],
                            knT[:],
                            start=True, stop=True,
                        )
                    sl = slice(g * TPG * P, (g + 1) * TPG * P)
                    nc.scalar.activation(s1T[:, sl], argp[:], Act.Sigmoid)
                    if g % 2 == 1:
                        # sd for two groups at once (DVE, halves)
                        h = g // 2
                        hl = slice(h * 2 * TPG * P, (h + 1) * 2 * TPG * P)
                        nc.vector.affine_mul_reduce(
                            sdT[:, hl], sdacc[:, h:h + 1], s1T[:, hl],
                            s1T[:, hl], -1.0, 1.0,
                        )

                # ---- phase 2: conv products + dots ----
                HT = TILES // 2  # tiles per half (8)
                for qd_ in range(GROUPS):
                    qq = ps_qq.tile([P, TPG * QPITCH], dt, tag="qq")
                    for i in range(TPG):
                        t = qd_ * TPG + i
                        base = i * QPITCH
                        nc.tensor.matmul(
                            qq[:, base:base + T],
                            s1T[:, t * P:(t + 1) * P], m2tl[:],
                            start=True, stop=True,
                        )
                        nc.tensor.matmul(
                            qq[:, base + T:base + 3 * T],
                            sdT[:, t * P:(t + 1) * P], muvl[:],
                            start=True, stop=True,
                        )
                    # one PSUM->SBUF bf16 copy for the whole quad (Scalar)
                    qsq = qsall[:, qd_ * 3 * T * TPG:(qd_ + 1) * 3 * T * TPG]
                    qq_v = qq[:].rearrange("p (t c) -> p t c", t=TPG)[:, :, 0:3 * T]
                    nc.scalar.copy(qsq.rearrange("p (t c) -> p t c", t=TPG), qq_v)

                    if qd_ % 2 == 1:
                        # r2 for a half (8 tiles): 2 batched DVE ops
                        h = qd_ // 2
                        hq = slice(h * HT * 3 * T, (h + 1) * HT * 3 * T)
                        hr = slice(h * HT * T, (h + 1) * HT * T)
                        qv = (qsall[:, hq]
                              .rearrange("p (t c) -> p t c", t=HT)[:, :, 0:T])
                        a2b = (a2c[:, h * HT:(h + 1) * HT]
                               .unsqueeze(2).broadcast_to([P, HT, T]))
                        r2t_v = r2tmp[:, hr].rearrange("p (t j) -> p t j", t=HT)
                        nc.vector.tensor_tensor(r2t_v, qv, a2b, Alu.mult)
                        nc.vector.tensor_tensor(
                            r2all[:, hr], r2tmp[:, hr], nctc[:, hr], Alu.add)
                        # prods for the two quads of this half
                        for qq2 in (qd_ - 1, qd_):
                            prodq = prodall[:, qq2 * 3 * T * TPG:
                                            (qq2 + 1) * 3 * T * TPG]
                            r2b = (r2all[:, qq2 * TPG * T:(qq2 + 1) * TPG * T]
                                   .rearrange("p (t j) -> p t j", t=TPG)
                                   .unsqueeze(2).broadcast_to([P, TPG, 3, T]))
                            qs4 = (qsall[:, qq2 * 3 * T * TPG:
                                         (qq2 + 1) * 3 * T * TPG]
                                   .rearrange("p (t s j) -> p t s j",
                                              t=TPG, s=3))
                            eng = nc.gpsimd if (qq2 % 2 == 0) else nc.vector
                            eng.tensor_tensor(
                                prodq.rearrange("p (t s j) -> p t s j",
                                                t=TPG, s=3),
                                qs4, r2b, Alu.mult)
                        # segmented reduce for the half: [P, 24, 64] -> [P, 24]
                        nc.vector.tensor_reduce(
                            accAll[:, h * 3 * HT:(h + 1) * 3 * HT],
                            prodall[:, h * HT * 3 * T:(h + 1) * HT * 3 * T]
                            .rearrange("p (ts j) -> p ts j", j=T),
                            Ax.X, Alu.add,
                        )

                # ---- combine: eta <- eta*s48 - LR*G48 + m48 + cpl48 ----
                # accAll columns: 3*t + {0:gA, 1:U, 2:V}
                gAv = accAll[:, 0:3 * TILES:3]
                Uv = accAll[:, 1:3 * TILES:3]
                Vv = accAll[:, 2:3 * TILES:3]
                G48 = mpool.tile([P, 3 * TILES], dt, tag="G48")
                nc.gpsimd.tensor_copy(G48[:, 0:TILES], gAv)
                p1 = mpool.tile([P, TILES], dt, tag="p1")
                nc.gpsimd.tensor_tensor(p1[:], eA, Uv, Alu.mult)
                p2 = mpool.tile([P, TILES], dt, tag="p2")
                nc.gpsimd.tensor_tensor(p2[:], eA, Vv, Alu.mult)
                wk = mpool.tile([P, TILES], dt, tag="wk")
                nc.gpsimd.tensor_tensor(wk[:], eT, p1[:], Alu.mult)
                nc.gpsimd.tensor_tensor(G48[:, TILES:2 * TILES], wk[:], p2[:],
                                        Alu.subtract)
                nc.gpsimd.tensor_tensor(G48[:, 2 * TILES:3 * TILES], p1[:], eK,
                                        Alu.mult)
                # update chain
                m48 = mpool.tile([P, 3 * TILES], dt, tag="m48")
                nc.gpsimd.tensor_scalar(m48[:], eta48[:], 0.0, -2.0 * LR,
                                        Alu.min, Alu.mult)
                t1 = mpool.tile([P, 3 * TILES], dt, tag="t1")
                nc.vector.scalar_tensor_tensor(t1[:], G48[:], -LR, cpl48[:],
                                               Alu.mult, Alu.add)
                t2 = mpool.tile([P, 3 * TILES], dt, tag="t2")
                nc.gpsimd.tensor_tensor(t2[:], eta48[:], s48[:], Alu.mult)
                t3 = mpool.tile([P, 3 * TILES], dt, tag="t3")
                nc.vector.tensor_tensor(t3[:], t1[:], m48[:], Alu.add)
                eta48n = spool.tile([P, 3 * TILES], dt, tag="eta48")
                nc.vector.tensor_tensor(eta48n[:], t2[:], t3[:], Alu.add)
                eta48 = eta48n

            nc.gpsimd.dma_start(d_out[:], eta48[:])

    nc.finalize()
    _NC_CACHE["nc"] = nc
    return nc


# ---------------------------------------------------------------------------
# public entry point
# ---------------------------------------------------------------------------

def _make_in_maps(ctc, aif, time, eta_nn, lambda_reg):
    f32 = np.float32
    M2L, M2VL, tau, ctc_dc, C_dc, creg = _preprocess(
        ctc, aif, time, eta_nn, lambda_reg)

    toc = 2.0 / C_dc
    sA, sK, sT0 = (1.0 - LR * creg).astype(np.float64)

    import ml_dtypes
    bf16 = ml_dtypes.bfloat16
    tauf = tau.astype(np.float32)
    # argw[2t, t*S+s] = 1 ; argw[2t+1, t*S+s] = tau_s
    argw = np.zeros((2 * TILES, TILES * S), bf16)
    for t_ in range(TILES):
        argw[2 * t_, t_ * S:(t_ + 1) * S] = 1.0
        argw[2 * t_ + 1, t_ * S:(t_ + 1) * S] = tauf
    ident = np.eye(P, dtype=bf16)
    m2tl = np.ascontiguousarray(M2L.T).astype(bf16)        # [S, 64]
    muvl = np.zeros((S, 2 * T), bf16)
    muvl[:, 0:T] = M2L.T
    muvl[:, T:2 * T] = M2VL.T

    consts = np.full((P, TILES), toc, f32)
    s48 = np.zeros((P, 3 * TILES), f32)
    s48[:, 0:TILES] = sA
    s48[:, TILES:2 * TILES] = sK
    s48[:, 2 * TILES:] = sT0

    in_maps = []
    for m in range(N_CORES):
        rows = slice(m * ROWS_PER_CORE, (m + 1) * ROWS_PER_CORE)
        cd = ctc_dc[rows]                     # [16, 128, 64]
        negctc2 = np.ascontiguousarray(
            (-toc * cd).transpose(1, 0, 2).reshape(P, TILES * T)).astype(bf16)
        pr = eta_nn[0, :, rows, :].astype(np.float64)   # [3, 16, 128]
        eta0 = np.ascontiguousarray(
            pr.transpose(2, 0, 1).reshape(P, 3 * TILES)).astype(f32)
        cpl48 = np.zeros((P, 3 * TILES), f32)
        for c in range(3):
            cpl48[:, c * TILES:(c + 1) * TILES] = (LR * creg[c] * pr[c]).T
        in_maps.append({
            "argw": argw, "ident": ident, "m2tl": m2tl, "muvl": muvl,
            "negctc2": negctc2, "eta0": eta0, "cpl48": cpl48, "s48": s48,
            "consts": consts,
        })
    return in_maps


def kernel(ctc, aif, time, seg, eta_nn, lambda_reg):
    from concourse.bass_utils import run_bass_kernel_spmd

    ctc = np.asarray(ctc)
    aif = np.asarray(aif)
    time = np.asarray(time)
    eta_nn = np.asarray(eta_nn)
    lambda_reg = np.asarray(lambda_reg)

    in_maps = _make_in_maps(ctc, aif, time, eta_nn, lambda_reg)
    nc = _build_nc()
    res = run_bass_kernel_spmd(nc, in_maps, list(range(N_CORES)))

    out = np.zeros((1, 3, H, W), np.float32)
    for m in range(N_CORES):
        rows = slice(m * ROWS_PER_CORE, (m + 1) * ROWS_PER_CORE)
        arr = res.results[m]["out"]                  # [128, 48]
        out[0, :, rows, :] = arr.reshape(P, 3, TILES).transpose(1, 2, 0)
    return out


# revision 11
# speedup vs baseline: 1.1560x; 1.1560x over previous
"""Trainium2 Bass kernel for the DeepFermi deconvolution GD problem (v2).

Reference: 10 fixed-step GD iterations of a per-pixel objective

    F(eta) = ||ctc_dc - conv(aif_os, fermi_ir(eta))[::8]/8||^2 / C_dc
             + softplus(lambda) * ||(eta - eta_nn)||^2_Cnn + ||relu(-eta)||^2

The time-axis convolution with the fixed AIF is a 64x512 matrix M2 (the sharp
C=500 onset step is folded into it).  The per-pixel factor sigmoid(k*(t0-tsh))
is smooth, so we sample it on an S=64 uniform grid tau and fold the 512->S
linear interpolation into the fixed matrices:

    M2L  = M2 @ L            [64, S]
    M2VL = M2L * tau         [64, S]
    s1_s = sigmoid(k*(t0 - tau_s))     sd_s = s1_s*(1-s1_s)
    q    = M2L @ s1;  qd = M2L @ sd;  qdv = M2VL @ sd
    r2   = (2/C_dc)*(A*q - ctc_dc)
    gA   = r2.q;  U = r2.qd;  V = r2.qdv
    gk   = A*(t0*U - V);  gt0 = A*k*U

(numpy-validated: rel err ~2e-5 vs the 512-point reference, tolerance 2e-2).

Layout: H rows sharded over 8 cores (16 rows = 16 tiles of 128 pixels each).
Time-major [S, pixels] for sigmoid/sd (batched over 4-tile groups), pixel-major
[pixels, j] for the conv outputs.  Conv outputs land in 2-bank quad PSUM tiles
(4 tiles, 256-col pitch) so the PSUM->SBUF copy is one Scalar op per quad and
the dot products are one product op + one segmented tensor_reduce per quad.
"""

import numpy as np

OSAMP = 8
MAX_ITER = 10
NEG_SHIFT = 2 * OSAMP
OTP = 5
C_SHARP = 500.0
LR = 0.1
T = 64
TOS = OSAMP * T  # 512
S = 64           # reduced time-sample grid for the smooth sigmoid
H = 128
W = 128
N_CORES = 8
ROWS_PER_CORE = H // N_CORES  # 16
TILES = ROWS_PER_CORE
P = 128
GROUPS = 4
TPG = TILES // GROUPS  # tiles per group (4)
QPITCH = 256           # per-tile column pitch inside a quad PSUM tile


# ---------------------------------------------------------------------------
# host-side math (iteration independent)
# ---------------------------------------------------------------------------

def _resize_mat(in_size, out_size):
    scale = out_size / in_size
    sample_f = (np.arange(out_size) + 0.5) / scale - 0.5
    x = np.abs(sample_f[None, :] - np.arange(in_size)[:, None])
    w = np.maximum(0.0, 1.0 - x)
    tot = w.sum(0, keepdims=True)
    w = np.where(np.abs(tot) > 1e-4, w / tot, 0.0)
    return w  # float64


def _sigmoid(x):
    return 1.0 / (1.0 + np.exp(-np.clip(x, -500, 500)))


def _preprocess(ctc, aif, time, eta_nn, lambda_reg):
    f64 = np.float64
    R = _resize_mat(T, TOS)
    aif0 = (aif.astype(f64) - aif.astype(f64)[..., :OTP].mean(-1, keepdims=True))
    ctc0 = (ctc.astype(f64) - ctc.astype(f64)[..., :OTP].mean(-1, keepdims=True))
    aif_os = (aif0 @ R)[0, 0, 0]                    # [512]
    t_os = time.astype(f64) @ R                     # [512]
    ctc_dc = (ctc0 @ R[:, ::OSAMP])[0]              # [H,W,64]
    C_dc = float((ctc_dc.astype(np.float32) ** 2).sum(dtype=np.float64))
    tsh = t_os - t_os[NEG_SHIFT]
    s2 = _sigmoid((C_SHARP * tsh).astype(np.float32).astype(f64))
    idx = NEG_SHIFT + 8 * np.arange(T)[:, None] - np.arange(TOS)[None, :]
    valid = (idx >= 0) & (idx <= TOS - 1)
    M = np.where(valid, aif_os[np.clip(idx, 0, TOS - 1)], 0.0) / OSAMP  # [64,512]
    M2 = M * s2[None, :]
    # S-point grid in tsh-space + hat-function interpolation matrix L
    tau = np.linspace(tsh.min(), tsh.max(), S)
    dt_ = tau[1] - tau[0]
    pos = (tsh - tau[0]) / dt_
    i0 = np.clip(np.floor(pos).astype(int), 0, S - 2)
    frac = np.clip(pos - i0, 0.0, 1.0)
    L = np.zeros((TOS, S))
    L[np.arange(TOS), i0] = 1 - frac
    L[np.arange(TOS), i0 + 1] = frac
    M2L = M2 @ L                                    # [64, S]
    M2VL = M2L * tau[None, :]
    C_nn = (eta_nn.astype(f64) ** 2).sum(axis=(0, 2, 3))  # [3]
    sp_lam = np.logaddexp(0.0, float(lambda_reg.reshape(-1)[0]))
    creg = 2.0 * sp_lam / C_nn                      # [3]
    return M2L, M2VL, tau, ctc_dc, C_dc, creg


# ---------------------------------------------------------------------------
# bass module (input-value independent; all data arrives via DRAM tensors)
# ---------------------------------------------------------------------------

_NC_CACHE = {}


def _build_nc():
    if "nc" in _NC_CACHE:
        return _NC_CACHE["nc"]

    import concourse.mybir as mybir
    import concourse.tile as tile
    from concourse import bacc

    dt = mybir.dt.float32
    bf = mybir.dt.bfloat16
    Alu = mybir.AluOpType
    Act = mybir.ActivationFunctionType
    Ax = mybir.AxisListType

    nc = bacc.Bacc("TRN2", target_bir_lowering=False, debug=False)

    # shared constants (identical on every core)
    d_argw = nc.declare_dram_parameter("argw", [2 * TILES, TILES * S], bf,
                                       isOutput=False)
    d_ident = nc.declare_dram_parameter("ident", [P, P], bf, isOutput=False)
    d_m2tl = nc.declare_dram_parameter("m2tl", [S, T], bf, isOutput=False)
    d_muvl = nc.declare_dram_parameter("muvl", [S, 2 * T], bf, isOutput=False)
    d_s48 = nc.declare_dram_parameter("s48", [P, 3 * TILES], dt, isOutput=False)
    d_consts = nc.declare_dram_parameter("consts", [P, TILES], dt, isOutput=False)
    # per-core data
    d_nctc = nc.declare_dram_parameter("negctc2", [P, TILES * T], bf, isOutput=False)
    d_eta0 = nc.declare_dram_parameter("eta0", [P, 3 * TILES], dt, isOutput=False)
    d_cpl48 = nc.declare_dram_parameter("cpl48", [P, 3 * TILES], dt, isOutput=False)
    d_out = nc.declare_dram_parameter("out", [P, 3 * TILES], dt, isOutput=True)

    with tile.TileContext(nc) as tc:
        with (
            tc.tile_pool(name="const", bufs=1) as cpool,
            tc.tile_pool(name="state", bufs=2) as spool,
            tc.tile_pool(name="small", bufs=2) as mpool,
            tc.tile_pool(name="ps_arg", bufs=2, space="PSUM") as ps_arg,
            tc.tile_pool(name="ps_qq", bufs=2, space="PSUM") as ps_qq,
            tc.tile_pool(name="ps_k", bufs=1, space="PSUM") as ps_k,
        ):
            # ---- load constants ----
            argw = cpool.tile([2 * TILES, TILES * S], bf, tag="argw")
            nc.gpsimd.dma_start(argw[:], d_argw[:])
            ident = cpool.tile([P, P], bf, tag="ident")
            nc.gpsimd.dma_start(ident[:], d_ident[:])
            m2tl = cpool.tile([S, T], bf, tag="m2tl")
            nc.gpsimd.dma_start(m2tl[:], d_m2tl[:])
            muvl = cpool.tile([S, 2 * T], bf, tag="muvl")
            nc.gpsimd.dma_start(muvl[:], d_muvl[:])
            nctc = cpool.tile([P, TILES * T], bf, tag="nctc")
            nc.gpsimd.dma_start(nctc[:], d_nctc[:])
            cpl48 = cpool.tile([P, 3 * TILES], dt, tag="cpl48")
            nc.gpsimd.dma_start(cpl48[:], d_cpl48[:])
            s48 = cpool.tile([P, 3 * TILES], dt, tag="s48")
            nc.gpsimd.dma_start(s48[:], d_s48[:])
            consts = cpool.tile([P, TILES], dt, tag="consts")
            nc.gpsimd.dma_start(consts[:], d_consts[:])
            eta_in = cpool.tile([P, 3 * TILES], dt, tag="eta_in")
            nc.gpsimd.dma_start(eta_in[:], d_eta0[:])

            # persistent work buffers
            s1T = cpool.tile([S, TILES * P], bf, tag="s1T")
            sdT = cpool.tile([S, TILES * P], bf, tag="sdT")
            sdtmp = cpool.tile([S, TILES * P], bf, tag="sdtmp")
            qall = cpool.tile([P, TILES * T], bf, tag="qall")
            qdall = cpool.tile([P, TILES * T], bf, tag="qdall")
            qdvall = cpool.tile([P, TILES * T], bf, tag="qdvall")
            r2all = cpool.tile([P, TILES * T], bf, tag="r2all")
            r2tmp = cpool.tile([P, TILES * T], bf, tag="r2tmp")
            prodA = cpool.tile([P, TILES * T], bf, tag="prodA")
            prodU = cpool.tile([P, TILES * T], bf, tag="prodU")
            prodV = cpool.tile([P, TILES * T], bf, tag="prodV")
            accU = cpool.tile([P, TILES], dt, tag="accU")
            accV = cpool.tile([P, TILES], dt, tag="accV")

            eta48 = spool.tile([P, 3 * TILES], dt, tag="eta48")
            nc.vector.tensor_copy(eta48[:], eta_in[:])

            for it in range(MAX_ITER):
                eA = eta48[:, 0:TILES]
                eK = eta48[:, TILES:2 * TILES]
                eT = eta48[:, 2 * TILES:3 * TILES]

                # ---- derived per-iteration tensors ----
                kn = spool.tile([P, 2 * TILES], bf, tag="kn")
                nc.gpsimd.tensor_tensor(kn[:, 0:2 * TILES:2], eK, eT, Alu.mult)
                nc.gpsimd.tensor_scalar_mul(kn[:, 1:2 * TILES:2], eK, -1.0)
                knt_ps = ps_k.tile([2 * TILES, P], bf, tag="kntp")
                nc.tensor.transpose(knt_ps[:], kn[:], ident[:])
                knT = spool.tile([2 * TILES, P], bf, tag="knT")
                nc.scalar.copy(knT[:], knt_ps[:])
                # a2c = (2/C_dc) * A
                a2c = spool.tile([P, TILES], dt, tag="a2c")
                nc.gpsimd.tensor_tensor(a2c[:], eA, consts[:], Alu.mult)

                # ---- phase 1: argT -> sigmoid -> sd (4-tile groups) ----
                for g in range(GROUPS):
                    argp = ps_arg.tile([S, TPG * P], dt, tag="argp")
                    for tt in range(TPG):
                        t = g * TPG + tt
                        nc.tensor.matmul(
                            argp[:, tt * P:(tt + 1) * P],
                            argw[:, t * S:(t + 1) * S],
                            knT[:],
                            start=True, stop=True,
                        )
                    sl = slice(g * TPG * P, (g + 1) * TPG * P)
                    nc.scalar.activation(s1T[:, sl], argp[:], Act.Sigmoid)
                    if g % 2 == 1:
                        # sd = (1 - s1)*s1 for two groups (dense 4x + 2x ops)
                        h = g // 2
                        hl = slice(h * 2 * TPG * P, (h + 1) * 2 * TPG * P)
                        nc.vector.tensor_scalar(
                            sdtmp[:, hl], s1T[:, hl], -1.0, 1.0,
                            Alu.mult, Alu.add)
                        nc.vector.tensor_tensor(
                            sdT[:, hl], sdtmp[:, hl], s1T[:, hl], Alu.mult)

                G48 = mpool.tile([P, 3 * TILES], dt, tag="G48")

                # ---- phase 2: conv outputs + dot products ----
                HT = TILES // 2  # tiles per half (8)
                for qd_ in range(GROUPS):
                    qq = ps_qq.tile([P, TPG * QPITCH], dt, tag="qq")
                    for i in range(TPG):
                        t = qd_ * TPG + i
                        base = i * QPITCH
                        nc.tensor.matmul(
                            qq[:, base:base + T],
                            s1T[:, t * P:(t + 1) * P], m2tl[:],
                            start=True, stop=True,
                        )
                        nc.tensor.matmul(
                            qq[:, base + T:base + 3 * T],
                            sdT[:, t * P:(t + 1) * P], muvl[:],
                            start=True, stop=True,
                        )
                    # PSUM->SBUF bf16 slot copies (dense destinations)
                    qsl = slice(qd_ * TPG * T, (qd_ + 1) * TPG * T)
                    qq3 = qq[:].rearrange("p (t c) -> p t c", t=TPG)
                    nc.scalar.copy(
                        qall[:, qsl].rearrange("p (t j) -> p t j", t=TPG),
                        qq3[:, :, 0:T])
                    nc.scalar.copy(
                        qdall[:, qsl].rearrange("p (t j) -> p t j", t=TPG),
                        qq3[:, :, T:2 * T])
                    nc.scalar.copy(
                        qdvall[:, qsl].rearrange("p (t j) -> p t j", t=TPG),
                        qq3[:, :, 2 * T:3 * T])

                    if qd_ % 2 == 1:
                        # per half: r2, three dense products, three reduces
                        h = qd_ // 2
                        hr = slice(h * HT * T, (h + 1) * HT * T)
                        a2b = (a2c[:, h * HT:(h + 1) * HT]
                               .unsqueeze(2).broadcast_to([P, HT, T]))
                        nc.vector.tensor_tensor(
                            r2tmp[:, hr].rearrange("p (t j) -> p t j", t=HT),
                            qall[:, hr].rearrange("p (t j) -> p t j", t=HT),
                            a2b, Alu.mult)
                        nc.vector.tensor_tensor(
                            r2all[:, hr], r2tmp[:, hr], nctc[:, hr], Alu.add)
                        nc.vector.tensor_tensor(
                            prodA[:, hr], qall[:, hr], r2all[:, hr], Alu.mult)
                        nc.vector.tensor_tensor(
                            prodU[:, hr], qdall[:, hr], r2all[:, hr], Alu.mult)
                        nc.vector.tensor_tensor(
                            prodV[:, hr], qdvall[:, hr], r2all[:, hr], Alu.mult)
                        hsl = slice(h * HT, (h + 1) * HT)
                        nc.vector.tensor_reduce(
                            G48[:, h * HT:(h + 1) * HT],
                            prodA[:, hr].rearrange("p (t j) -> p t j", t=HT),
                            Ax.X, Alu.add,
                        )
                        nc.vector.tensor_reduce(
                            accU[:, hsl],
                            prodU[:, hr].rearrange("p (t j) -> p t j", t=HT),
                            Ax.X, Alu.add,
                        )
                        nc.vector.tensor_reduce(
                            accV[:, hsl],
                            prodV[:, hr].rearrange("p (t j) -> p t j", t=HT),
                            Ax.X, Alu.add,
                        )

                # ---- combine: eta <- eta*s48 - LR*G48 + m48 + cpl48 ----
                p1 = mpool.tile([P, TILES], dt, tag="p1")
                nc.gpsimd.tensor_tensor(p1[:], eA, accU[:], Alu.mult)
                p2 = mpool.tile([P, TILES], dt, tag="p2")
                nc.gpsimd.tensor_tensor(p2[:], eA, accV[:], Alu.mult)
                wk = mpool.tile([P, TILES], dt, tag="wk")
                nc.gpsimd.tensor_tensor(wk[:], eT, p1[:], Alu.mult)
                nc.gpsimd.tensor_tensor(G48[:, TILES:2 * TILES], wk[:], p2[:],
                                        Alu.subtract)
                nc.gpsimd.tensor_tensor(G48[:, 2 * TILES:3 * TILES], p1[:], eK,
                                        Alu.mult)
                # update chain
                m48 = mpool.tile([P, 3 * TILES], dt, tag="m48")
                nc.gpsimd.tensor_scalar(m48[:], eta48[:], 0.0, -2.0 * LR,
                                        Alu.min, Alu.mult)
                t1 = mpool.tile([P, 3 * TILES], dt, tag="t1")
                nc.vector.scalar_tensor_tensor(t1[:], G48[:], -LR, cpl48[:],
                                               Alu.mult, Alu.add)
                t2 = mpool.tile([P, 3 * TILES], dt, tag="t2")
                nc.gpsimd.tensor_tensor(t2[:], eta48[:], s48[:], Alu.mult)
                t3 = mpool.tile([P, 3 * TILES], dt, tag="t3")
                nc.vector.tensor_tensor(t3[:], t1[:], m48[:], Alu.add)
                eta48n = spool.tile([P, 3 * TILES], dt, tag="eta48")
                nc.vector.tensor_tensor(eta48n[:], t2[:], t3[:], Alu.add)
                eta48 = eta48n

            nc.gpsimd.dma_start(d_out[:], eta48[:])

    nc.finalize()
    _NC_CACHE["nc"] = nc
    return nc


# ---------------------------------------------------------------------------
# public entry point
# ---------------------------------------------------------------------------

def _make_in_maps(ctc, aif, time, eta_nn, lambda_reg):
    f32 = np.float32
    M2L, M2VL, tau, ctc_dc, C_dc, creg = _preprocess(
        ctc, aif, time, eta_nn, lambda_reg)

    toc = 2.0 / C_dc
    sA, sK, sT0 = (1.0 - LR * creg).astype(np.float64)

    import ml_dtypes
    bf16 = ml_dtypes.bfloat16
    tauf = tau.astype(np.float32)
    # argw[2t, t*S+s] = 1 ; argw[2t+1, t*S+s] = tau_s
    argw = np.zeros((2 * TILES, TILES * S), bf16)
    for t_ in range(TILES):
        argw[2 * t_, t_ * S:(t_ + 1) * S] = 1.0
        argw[2 * t_ + 1, t_ * S:(t_ + 1) * S] = tauf
    ident = np.eye(P, dtype=bf16)
    m2tl = np.ascontiguousarray(M2L.T).astype(bf16)        # [S, 64]
    muvl = np.zeros((S, 2 * T), bf16)
    muvl[:, 0:T] = M2L.T
    muvl[:, T:2 * T] = M2VL.T

    consts = np.full((P, TILES), toc, f32)
    s48 = np.zeros((P, 3 * TILES), f32)
    s48[:, 0:TILES] = sA
    s48[:, TILES:2 * TILES] = sK
    s48[:, 2 * TILES:] = sT0

    in_maps = []
    for m in range(N_CORES):
        rows = slice(m * ROWS_PER_CORE, (m + 1) * ROWS_PER_CORE)
        cd = ctc_dc[rows]                     # [16, 128, 64]
        negctc2 = np.ascontiguousarray(
            (-toc * cd).transpose(1, 0, 2).reshape(P, TILES * T)).astype(bf16)
        pr = eta_nn[0, :, rows, :].astype(np.float64)   # [3, 16, 128]
        eta0 = np.ascontiguousarray(
            pr.transpose(2, 0, 1).reshape(P, 3 * TILES)).astype(f32)
        cpl48 = np.zeros((P, 3 * TILES), f32)
        for c in range(3):
            cpl48[:, c * TILES:(c + 1) * TILES] = (LR * creg[c] * pr[c]).T
        in_maps.append({
            "argw": argw, "ident": ident, "m2tl": m2tl, "muvl": muvl,
            "negctc2": negctc2, "eta0": eta0, "cpl48": cpl48, "s48": s48,
            "consts": consts,
        })
    return in_maps


def kernel(ctc, aif, time, seg, eta_nn, lambda_reg):
    from concourse.bass_utils import run_bass_kernel_spmd

    ctc = np.asarray(ctc)
    aif = np.asarray(aif)
    time = np.asarray(time)
    eta_nn = np.asarray(eta_nn)
    lambda_reg = np.asarray(lambda_reg)

    in_maps = _make_in_maps(ctc, aif, time, eta_nn, lambda_reg)
    nc = _build_nc()
    res = run_bass_kernel_spmd(nc, in_maps, list(range(N_CORES)))

    out = np.zeros((1, 3, H, W), np.float32)
    for m in range(N_CORES):
        rows = slice(m * ROWS_PER_CORE, (m + 1) * ROWS_PER_CORE)
        arr = res.results[m]["out"]                  # [128, 48]
        out[0, :, rows, :] = arr.reshape(P, 3, TILES).transpose(1, 2, 0)
    return out


# revision 15
# speedup vs baseline: 1.2205x; 1.0559x over previous
"""Trainium2 Bass kernel for the DeepFermi deconvolution GD problem (v2).

Reference: 10 fixed-step GD iterations of a per-pixel objective

    F(eta) = ||ctc_dc - conv(aif_os, fermi_ir(eta))[::8]/8||^2 / C_dc
             + softplus(lambda) * ||(eta - eta_nn)||^2_Cnn + ||relu(-eta)||^2

The time-axis convolution with the fixed AIF is a 64x512 matrix M2 (the sharp
C=500 onset step is folded into it).  The per-pixel factor sigmoid(k*(t0-tsh))
is smooth, so we sample it on an S=64 uniform grid tau and fold the 512->S
linear interpolation into the fixed matrices:

    M2L  = M2 @ L            [64, S]
    M2VL = M2L * tau         [64, S]
    s1_s = sigmoid(k*(t0 - tau_s))     sd_s = s1_s*(1-s1_s)
    q    = M2L @ s1;  qd = M2L @ sd;  qdv = M2VL @ sd
    r2   = (2/C_dc)*(A*q - ctc_dc)
    gA   = r2.q;  U = r2.qd;  V = r2.qdv
    gk   = A*(t0*U - V);  gt0 = A*k*U

(numpy-validated: rel err ~2e-5 vs the 512-point reference, tolerance 2e-2).

Layout: H rows sharded over 8 cores (16 rows = 16 tiles of 128 pixels each).
Time-major [S, pixels] for sigmoid/sd (batched over 4-tile groups), pixel-major
[pixels, j] for the conv outputs.  Conv outputs land in 2-bank quad PSUM tiles
(4 tiles, 256-col pitch) so the PSUM->SBUF copy is one Scalar op per quad and
the dot products are one product op + one segmented tensor_reduce per quad.
"""

import numpy as np

OSAMP = 8
MAX_ITER = 10
NEG_SHIFT = 2 * OSAMP
OTP = 5
C_SHARP = 500.0
LR = 0.1
T = 64
TOS = OSAMP * T  # 512
S = 64           # reduced time-sample grid for the smooth sigmoid
H = 128
W = 128
N_CORES = 8
ROWS_PER_CORE = H // N_CORES  # 16
TILES = ROWS_PER_CORE
P = 128
GROUPS = 4
TPG = TILES // GROUPS  # tiles per group (4)
QPITCH = 256           # per-tile column pitch inside a quad PSUM tile


# ---------------------------------------------------------------------------
# host-side math (iteration independent)
# ---------------------------------------------------------------------------

def _resize_mat(in_size, out_size):
    scale = out_size / in_size
    sample_f = (np.arange(out_size) + 0.5) / scale - 0.5
    x = np.abs(sample_f[None, :] - np.arange(in_size)[:, None])
    w = np.maximum(0.0, 1.0 - x)
    tot = w.sum(0, keepdims=True)
    w = np.where(np.abs(tot) > 1e-4, w / tot, 0.0)
    return w  # float64


def _sigmoid(x):
    return 1.0 / (1.0 + np.exp(-np.clip(x, -500, 500)))


def _preprocess(ctc, aif, time, eta_nn, lambda_reg):
    f64 = np.float64
    R = _resize_mat(T, TOS)
    aif0 = (aif.astype(f64) - aif.astype(f64)[..., :OTP].mean(-1, keepdims=True))
    ctc0 = (ctc.astype(f64) - ctc.astype(f64)[..., :OTP].mean(-1, keepdims=True))
    aif_os = (aif0 @ R)[0, 0, 0]                    # [512]
    t_os = time.astype(f64) @ R                     # [512]
    ctc_dc = (ctc0 @ R[:, ::OSAMP])[0]              # [H,W,64]
    C_dc = float((ctc_dc.astype(np.float32) ** 2).sum(dtype=np.float64))
    tsh = t_os - t_os[NEG_SHIFT]
    s2 = _sigmoid((C_SHARP * tsh).astype(np.float32).astype(f64))
    idx = NEG_SHIFT + 8 * np.arange(T)[:, None] - np.arange(TOS)[None, :]
    valid = (idx >= 0) & (idx <= TOS - 1)
    M = np.where(valid, aif_os[np.clip(idx, 0, TOS - 1)], 0.0) / OSAMP  # [64,512]
    M2 = M * s2[None, :]
    # S-point grid in tsh-space + hat-function interpolation matrix L
    tau = np.linspace(tsh.min(), tsh.max(), S)
    dt_ = tau[1] - tau[0]
    pos = (tsh - tau[0]) / dt_
    i0 = np.clip(np.floor(pos).astype(int), 0, S - 2)
    frac = np.clip(pos - i0, 0.0, 1.0)
    L = np.zeros((TOS, S))
    L[np.arange(TOS), i0] = 1 - frac
    L[np.arange(TOS), i0 + 1] = frac
    M2L = M2 @ L                                    # [64, S]
    M2VL = M2L * tau[None, :]
    C_nn = (eta_nn.astype(f64) ** 2).sum(axis=(0, 2, 3))  # [3]
    sp_lam = np.logaddexp(0.0, float(lambda_reg.reshape(-1)[0]))
    creg = 2.0 * sp_lam / C_nn                      # [3]
    return M2L, M2VL, tau, ctc_dc, C_dc, creg


# ---------------------------------------------------------------------------
# bass module (input-value independent; all data arrives via DRAM tensors)
# ---------------------------------------------------------------------------

_NC_CACHE = {}


def _build_nc():
    if "nc" in _NC_CACHE:
        return _NC_CACHE["nc"]

    import concourse.mybir as mybir
    import concourse.tile as tile
    from concourse import bacc

    dt = mybir.dt.float32
    bf = mybir.dt.bfloat16
    Alu = mybir.AluOpType
    Act = mybir.ActivationFunctionType
    Ax = mybir.AxisListType

    nc = bacc.Bacc("TRN2", target_bir_lowering=False, debug=False)

    # shared constants (identical on every core)
    d_argw = nc.declare_dram_parameter("argw", [TILES, TILES * S], bf,
                                       isOutput=False)
    d_ident = nc.declare_dram_parameter("ident", [P, P], bf, isOutput=False)
    d_m2tl = nc.declare_dram_parameter("m2tl", [S, T], bf, isOutput=False)
    d_muvl = nc.declare_dram_parameter("muvl", [S, 2 * T], bf, isOutput=False)
    d_s48 = nc.declare_dram_parameter("s48", [P, 3 * TILES], dt, isOutput=False)
    d_consts = nc.declare_dram_parameter("consts", [P, TILES], dt, isOutput=False)
    # per-core data
    d_nctc = nc.declare_dram_parameter("negctc2", [P, TILES * T], bf, isOutput=False)
    d_eta0 = nc.declare_dram_parameter("eta0", [P, 3 * TILES], dt, isOutput=False)
    d_cpl48 = nc.declare_dram_parameter("cpl48", [P, 3 * TILES], dt, isOutput=False)
    d_out = nc.declare_dram_parameter("out", [P, 3 * TILES], dt, isOutput=True)

    with tile.TileContext(nc) as tc:
        with (
            tc.tile_pool(name="const", bufs=1) as cpool,
            tc.tile_pool(name="state", bufs=4) as spool,
            tc.tile_pool(name="small", bufs=2) as mpool,
            tc.tile_pool(name="ps_arg", bufs=2, space="PSUM") as ps_arg,
            tc.tile_pool(name="ps_qq", bufs=2, space="PSUM") as ps_qq,
            tc.tile_pool(name="ps_k", bufs=2, space="PSUM") as ps_k,
        ):
            # ---- load constants ----
            argw = cpool.tile([TILES, TILES * S], bf, tag="argw")
            nc.gpsimd.dma_start(argw[:], d_argw[:])
            ident = cpool.tile([P, P], bf, tag="ident")
            nc.gpsimd.dma_start(ident[:], d_ident[:])
            m2tl = cpool.tile([S, T], bf, tag="m2tl")
            nc.gpsimd.dma_start(m2tl[:], d_m2tl[:])
            muvl = cpool.tile([S, 2 * T], bf, tag="muvl")
            nc.gpsimd.dma_start(muvl[:], d_muvl[:])
            nctc = cpool.tile([P, TILES * T], bf, tag="nctc")
            nc.gpsimd.dma_start(nctc[:], d_nctc[:])
            cpl48 = cpool.tile([P, 3 * TILES], dt, tag="cpl48")
            nc.gpsimd.dma_start(cpl48[:], d_cpl48[:])
            s48 = cpool.tile([P, 3 * TILES], dt, tag="s48")
            nc.gpsimd.dma_start(s48[:], d_s48[:])
            consts = cpool.tile([P, TILES], dt, tag="consts")
            nc.gpsimd.dma_start(consts[:], d_consts[:])
            eta_in = cpool.tile([P, 3 * TILES], dt, tag="eta_in")
            nc.gpsimd.dma_start(eta_in[:], d_eta0[:])

            # persistent work buffers
            s1T = cpool.tile([S, TILES * P], bf, tag="s1T")
            sdT = cpool.tile([S, TILES * P], bf, tag="sdT")
            sdtmp = cpool.tile([S, TILES * P], bf, tag="sdtmp")
            qall = cpool.tile([P, TILES * T], bf, tag="qall")
            qdall = cpool.tile([P, TILES * T], bf, tag="qdall")
            qdvall = cpool.tile([P, TILES * T], bf, tag="qdvall")
            r2all = cpool.tile([P, TILES * T], bf, tag="r2all")
            r2tmp = cpool.tile([P, TILES * T], bf, tag="r2tmp")
            prodA = cpool.tile([P, TILES * T], bf, tag="prodA")
            prodU = cpool.tile([P, TILES * T], bf, tag="prodU")
            prodV = cpool.tile([P, TILES * T], bf, tag="prodV")
            accU = cpool.tile([P, TILES], dt, tag="accU")
            accV = cpool.tile([P, TILES], dt, tag="accV")

            eta48 = spool.tile([P, 3 * TILES], dt, tag="eta48")
            nc.vector.tensor_copy(eta48[:], eta_in[:])

            HT = TILES // 2  # tiles per half (8)
            for it in range(MAX_ITER):
                eta48n = spool.tile([P, 3 * TILES], dt, tag="eta48")
                G48 = mpool.tile([P, 3 * TILES], dt, tag="G48")

                for h in range(2):
                    tsl = slice(h * HT, (h + 1) * HT)
                    eAh = eta48[:, h * HT:(h + 1) * HT]
                    eKh = eta48[:, TILES + h * HT:TILES + (h + 1) * HT]
                    eTh = eta48[:, 2 * TILES + h * HT:2 * TILES + (h + 1) * HT]

                    # ---- derived (per half) ----
                    kn = spool.tile([P, 2 * HT], bf, tag="kn")
                    nc.gpsimd.tensor_tensor(kn[:, 0:2 * HT:2], eKh, eTh,
                                            Alu.mult)
                    nc.gpsimd.tensor_scalar_mul(kn[:, 1:2 * HT:2], eKh, -1.0)
                    knt_ps = ps_k.tile([2 * HT, P], bf, tag="kntp")
                    nc.tensor.transpose(knt_ps[:], kn[:], ident[:])
                    knT = spool.tile([2 * HT, P], bf, tag="knT")
                    nc.scalar.copy(knT[:], knt_ps[:])
                    a2c = spool.tile([P, HT], dt, tag="a2c")
                    nc.gpsimd.tensor_tensor(a2c[:], eAh, consts[:, tsl],
                                            Alu.mult)

                    # ---- arg -> sigmoid -> sd (two 4-tile groups) ----
                    for g2 in range(2):
                        g = 2 * h + g2
                        argp = ps_arg.tile([S, TPG * P], dt, tag="argp")
                        for tt in range(TPG):
                            t = g * TPG + tt
                            nc.tensor.matmul(
                                argp[:, tt * P:(tt + 1) * P],
                                argw[:, t * S:(t + 1) * S],
                                knT[:],
                                start=True, stop=True,
                            )
                        sl = slice(g * TPG * P, (g + 1) * TPG * P)
                        nc.scalar.activation(s1T[:, sl], argp[:], Act.Sigmoid)
                        nc.vector.tensor_scalar(
                            sdtmp[:, sl], s1T[:, sl], -1.0, 1.0,
                            Alu.mult, Alu.add)
                        nc.vector.tensor_tensor(
                            sdT[:, sl], sdtmp[:, sl], s1T[:, sl], Alu.mult)

                    # ---- conv outputs (two quads) + slot copies ----
                    for g2 in range(2):
                        g = 2 * h + g2
                        qq = ps_qq.tile([P, TPG * QPITCH], dt, tag="qq")
                        for i in range(TPG):
                            t = g * TPG + i
                            base = i * QPITCH
                            nc.tensor.matmul(
                                qq[:, base:base + T],
                                s1T[:, t * P:(t + 1) * P], m2tl[:],
                                start=True, stop=True,
                            )
                            nc.tensor.matmul(
                                qq[:, base + T:base + 3 * T],
                                sdT[:, t * P:(t + 1) * P], muvl[:],
                                start=True, stop=True,
                            )
                        qsl = slice(g * TPG * T, (g + 1) * TPG * T)
                        qq3 = qq[:].rearrange("p (t c) -> p t c", t=TPG)
                        nc.scalar.copy(
                            qall[:, qsl].rearrange("p (t j) -> p t j", t=TPG),
                            qq3[:, :, 0:T])
                        nc.scalar.copy(
                            qdall[:, qsl].rearrange("p (t j) -> p t j", t=TPG),
                            qq3[:, :, T:2 * T])
                        nc.scalar.copy(
                            qdvall[:, qsl].rearrange("p (t j) -> p t j",
                                                     t=TPG),
                            qq3[:, :, 2 * T:3 * T])

                    # ---- dots for the half ----
                    hr = slice(h * HT * T, (h + 1) * HT * T)
                    a2b = a2c[:].unsqueeze(2).broadcast_to([P, HT, T])
                    nc.vector.tensor_tensor(
                        r2tmp[:, hr].rearrange("p (t j) -> p t j", t=HT),
                        qall[:, hr].rearrange("p (t j) -> p t j", t=HT),
                        a2b, Alu.mult)
                    nc.vector.tensor_tensor(
                        r2all[:, hr], r2tmp[:, hr], nctc[:, hr], Alu.add)
                    nc.gpsimd.tensor_tensor(
                        prodU[:, hr], qdall[:, hr], r2all[:, hr], Alu.mult)
                    nc.vector.tensor_tensor(
                        prodA[:, hr], qall[:, hr], r2all[:, hr], Alu.mult)
                    nc.vector.tensor_tensor(
                        prodV[:, hr], qdvall[:, hr], r2all[:, hr], Alu.mult)
                    nc.vector.tensor_reduce(
                        G48[:, h * HT:(h + 1) * HT],
                        prodA[:, hr].rearrange("p (t j) -> p t j", t=HT),
                        Ax.X, Alu.add,
                    )
                    nc.vector.tensor_reduce(
                        accU[:, tsl],
                        prodU[:, hr].rearrange("p (t j) -> p t j", t=HT),
                        Ax.X, Alu.add,
                    )
                    nc.vector.tensor_reduce(
                        accV[:, tsl],
                        prodV[:, hr].rearrange("p (t j) -> p t j", t=HT),
                        Ax.X, Alu.add,
                    )

                    # ---- combine (per half): eta' = eta*s48 - LR*G48 + m48
                    #      + cpl48, with gk = A*(t0*U - V), gt0 = A*k*U ----
                    p1 = mpool.tile([P, HT], dt, tag="p1")
                    nc.gpsimd.tensor_tensor(p1[:], eAh, accU[:, tsl], Alu.mult)
                    p2 = mpool.tile([P, HT], dt, tag="p2")
                    nc.gpsimd.tensor_tensor(p2[:], eAh, accV[:, tsl], Alu.mult)
                    wk = mpool.tile([P, HT], dt, tag="wk")
                    nc.gpsimd.tensor_tensor(wk[:], eTh, p1[:], Alu.mult)
                    nc.gpsimd.tensor_tensor(
                        G48[:, TILES + h * HT:TILES + (h + 1) * HT],
                        wk[:], p2[:], Alu.subtract)
                    nc.gpsimd.tensor_tensor(
                        G48[:, 2 * TILES + h * HT:2 * TILES + (h + 1) * HT],
                        p1[:], eKh, Alu.mult)
                    # strided [128, 3, HT] views of the three component blocks
                    ev = (eta48[:].rearrange("p (c t) -> p c t", c=3)
                          [:, :, h * HT:(h + 1) * HT])
                    env = (eta48n[:].rearrange("p (c t) -> p c t", c=3)
                           [:, :, h * HT:(h + 1) * HT])
                    gv = (G48[:].rearrange("p (c t) -> p c t", c=3)
                          [:, :, h * HT:(h + 1) * HT])
                    cplv = (cpl48[:].rearrange("p (c t) -> p c t", c=3)
                            [:, :, h * HT:(h + 1) * HT])
                    s48v = (s48[:].rearrange("p (c t) -> p c t", c=3)
                            [:, :, h * HT:(h + 1) * HT])
                    m48 = mpool.tile([P, 3 * HT], dt, tag="m48")
                    m48v = m48[:].rearrange("p (c t) -> p c t", c=3)
                    nc.vector.tensor_scalar(m48v, ev, 0.0, -2.0 * LR,
                                            Alu.min, Alu.mult)
                    t1 = mpool.tile([P, 3 * HT], dt, tag="t1")
                    t1v = t1[:].rearrange("p (c t) -> p c t", c=3)
                    nc.vector.scalar_tensor_tensor(t1v, gv, -LR, cplv,
                                                   Alu.mult, Alu.add)
                    t2 = mpool.tile([P, 3 * HT], dt, tag="t2")
                    t2v = t2[:].rearrange("p (c t) -> p c t", c=3)
                    nc.gpsimd.tensor_tensor(t2v, ev, s48v, Alu.mult)
                    t3 = mpool.tile([P, 3 * HT], dt, tag="t3")
                    nc.vector.tensor_tensor(t3[:], t1[:], m48[:], Alu.add)
                    nc.vector.tensor_tensor(env, t2[:].rearrange(
                        "p (c t) -> p c t", c=3), t3[:].rearrange(
                        "p (c t) -> p c t", c=3), Alu.add)

                eta48 = eta48n

            nc.gpsimd.dma_start(d_out[:], eta48[:])

    nc.finalize()
    _NC_CACHE["nc"] = nc
    return nc


# ---------------------------------------------------------------------------
# public entry point
# ---------------------------------------------------------------------------

def _make_in_maps(ctc, aif, time, eta_nn, lambda_reg):
    f32 = np.float32
    M2L, M2VL, tau, ctc_dc, C_dc, creg = _preprocess(
        ctc, aif, time, eta_nn, lambda_reg)

    toc = 2.0 / C_dc
    sA, sK, sT0 = (1.0 - LR * creg).astype(np.float64)

    import ml_dtypes
    bf16 = ml_dtypes.bfloat16
    tauf = tau.astype(np.float32)
    # per-half selectors: argw[2*(t%8), t*S+s] = 1 ; argw[2*(t%8)+1, .] = tau_s
    argw = np.zeros((TILES, TILES * S), bf16)
    for t_ in range(TILES):
        i_ = t_ % (TILES // 2)
        argw[2 * i_, t_ * S:(t_ + 1) * S] = 1.0
        argw[2 * i_ + 1, t_ * S:(t_ + 1) * S] = tauf
    ident = np.eye(P, dtype=bf16)
    m2tl = np.ascontiguousarray(M2L.T).astype(bf16)        # [S, 64]
    muvl = np.zeros((S, 2 * T), bf16)
    muvl[:, 0:T] = M2L.T
    muvl[:, T:2 * T] = M2VL.T

    consts = np.full((P, TILES), toc, f32)
    s48 = np.zeros((P, 3 * TILES), f32)
    s48[:, 0:TILES] = sA
    s48[:, TILES:2 * TILES] = sK
    s48[:, 2 * TILES:] = sT0

    in_maps = []
    for m in range(N_CORES):
        rows = slice(m * ROWS_PER_CORE, (m + 1) * ROWS_PER_CORE)
        cd = ctc_dc[rows]                     # [16, 128, 64]
        negctc2 = np.ascontiguousarray(
            (-toc * cd).transpose(1, 0, 2).reshape(P, TILES * T)).astype(bf16)
        pr = eta_nn[0, :, rows, :].astype(np.float64)   # [3, 16, 128]
        eta0 = np.ascontiguousarray(
            pr.transpose(2, 0, 1).reshape(P, 3 * TILES)).astype(f32)
        cpl48 = np.zeros((P, 3 * TILES), f32)
        for c in range(3):
            cpl48[:, c * TILES:(c + 1) * TILES] = (LR * creg[c] * pr[c]).T
        in_maps.append({
            "argw": argw, "ident": ident, "m2tl": m2tl, "muvl": muvl,
            "negctc2": negctc2, "eta0": eta0, "cpl48": cpl48, "s48": s48,
            "consts": consts,
        })
    return in_maps


def kernel(ctc, aif, time, seg, eta_nn, lambda_reg):
    from concourse.bass_utils import run_bass_kernel_spmd

    ctc = np.asarray(ctc)
    aif = np.asarray(aif)
    time = np.asarray(time)
    eta_nn = np.asarray(eta_nn)
    lambda_reg = np.asarray(lambda_reg)

    in_maps = _make_in_maps(ctc, aif, time, eta_nn, lambda_reg)
    nc = _build_nc()
    res = run_bass_kernel_spmd(nc, in_maps, list(range(N_CORES)))

    out = np.zeros((1, 3, H, W), np.float32)
    for m in range(N_CORES):
        rows = slice(m * ROWS_PER_CORE, (m + 1) * ROWS_PER_CORE)
        arr = res.results[m]["out"]                  # [128, 48]
        out[0, :, rows, :] = arr.reshape(P, 3, TILES).transpose(1, 2, 0)
    return out
